# revision 2
# baseline (speedup 1.0000x reference)
"""Dcls1d on 8 Trainium2 NeuronCores — bf16 weight-stationary version.

Problem (hardcoded): input [32, 256, 4096] f32, weight [256, 256, 7] f32,
P [256, 256, 7] f32, bias [256] f32 -> output [32, 256, 4097] f32.
Taps are scattered host-side into a dense [O, C, 53] kernel (positions
0..52 only — tap k's support is [8k-4, 8k+4], so 53..55 are all-zero),
then the conv runs on-device as 53 shifted [128x128]x[128x512] bf16
matmuls accumulating in PSUM fp32.

vs the fp32r baseline: bf16 streams at the same 1 col/cycle but allows
weight reuse across matmuls (fp32r forces an LDWEIGHTS per matmul and a
~20ns/MM handoff). Structure: phase = (chunk-group j, ot); 4 batches'
chunks share each loaded weight (4 PSUM banks round-robin), so the
weight load path is 4x amortized.

Sharding: data-parallel over batch — each of the 8 cores gets 4 batches.
"""

import os
from contextlib import ExitStack

import numpy as np

import concourse.bacc as bacc
import concourse.mybir as mybir
import concourse.tile as tile
from concourse.bass_utils import run_bass_kernel_spmd

DT = mybir.dt

B, CIN, COUT, L = 32, 256, 256, 4096
KTAPS, DIL, PAD = 7, 8, 28
LD = KTAPS * DIL  # 56 (reference's dense width)
ND = 53  # nonzero dense positions [0, 52]
TOUT = L + 2 * PAD - LD + 1  # 4097
NCORES = 8
BPC = B // NCORES  # batches per core

NT = 512  # output cols per psum tile
NCHUNK = L // NT  # 8 chunk groups
NA = 4  # aligned copies (bf16: 8B alignment needs offset % 4 == 0)
CH_W = NT + 56  # 568 xpad cols per chunk copy (max offset d-a = 52)

_nc_cache = {}


def build_dense_kernel(weight: np.ndarray, P: np.ndarray) -> np.ndarray:
    """Scatter taps into dense [O, C, LD] kernel. Replicates the reference's
    fp32 arithmetic exactly (clip/floor/frac all in float32)."""
    w = weight.astype(np.float32)
    pos = np.clip(P.astype(np.float32) + np.float32(LD // 2), np.float32(0.0), np.float32(LD - 1))
    lo = np.floor(pos)
    frac = pos - lo
    lo_i = lo.astype(np.int64)
    hi_i = np.minimum(lo_i + 1, LD - 1)
    O, C, K = w.shape
    oi = np.arange(O)[:, None, None]
    ci = np.arange(C)[None, :, None]
    D = np.zeros((O, C, LD), np.float32)
    np.add.at(D, (oi, ci, lo_i), w * (np.float32(1.0) - frac))
    np.add.at(D, (oi, ci, hi_i), w * frac)
    return D


def to_bf16(a: np.ndarray) -> np.ndarray:
    """Round fp32 -> bf16 (RNE), returned as an ml_dtypes.bfloat16 array."""
    bits = np.ascontiguousarray(a, np.float32).view(np.uint32)
    q = ((bits + np.uint32(0x7FFF) + ((bits >> np.uint32(16)) & np.uint32(1)))
         >> np.uint32(16)).astype(np.uint16)
    return q.view(mybir.dt.np(DT.bfloat16))


def build_nc(bpc=BPC, nchunk=NCHUNK):
    """Per-core program: conv of [bpc, 256, L'] with dense bf16 kernel."""
    Lc = nchunk * NT
    tout = Lc  # tail output col t=Lc is computed host-side

    nc = bacc.Bacc("TRN2", target_bir_lowering=False, debug=False)
    x_d = nc.dram_tensor("x", [bpc, CIN, Lc], DT.bfloat16, kind="ExternalInput").ap()
    dw_d = nc.dram_tensor("dw", [128, ND, 2, 2, 128], DT.bfloat16, kind="ExternalInput").ap()
    bias_d = nc.dram_tensor("bias", [128, 2], DT.float32, kind="ExternalInput").ap()
    zp_d = nc.dram_tensor("zp", [128, 2, 32], DT.bfloat16, kind="ExternalInput").ap()
    y_d = nc.dram_tensor("y", [bpc, COUT, tout], DT.float32, kind="ExternalOutput").ap()

    with ExitStack() as ctx:
        tc = ctx.enter_context(tile.TileContext(nc))
        wpool = ctx.enter_context(tc.tile_pool(name="w", bufs=1))
        xpool = ctx.enter_context(tc.tile_pool(name="x", bufs=2))
        opool = ctx.enter_context(tc.tile_pool(name="o", bufs=4))
        cpool = ctx.enter_context(tc.tile_pool(name="c", bufs=1))
        pspool = ctx.enter_context(tc.tile_pool(name="ps", bufs=2, space="PSUM"))

        # phases process d in a-class order (all d=a mod 4, then a+1, ...),
        # so weight DMA pieces are d-strided by 4: the a=0 class + the a=0
        # x-plane (~2MB total) make phase 0 compute-ready almost immediately.
        # a=0 sub-pieces go on the sync queue (race group-0 x on scalar);
        # classes a=1..3 are issued behind group-0's x on the scalar queue.
        dw_tiles = {}  # (a, sub) -> (tile, d_list)
        dw_dma = []  # late pieces: (tile, dram slice)
        for a in range(NA):
            for sub in range(4):
                ds = list(range(a + 16 * sub, min(ND, a + 16 * (sub + 1)), NA))
                if not ds:
                    continue
                t = wpool.tile(
                    [128, len(ds), 2, 2, 128], DT.bfloat16,
                    name=f"dw{a}_{sub}", tag=f"dw{a}_{sub}",
                )
                src = dw_d[:, ds[0] : ds[-1] + 1 : NA]
                if a == 0:
                    nc.sync.dma_start(t[:], src)
                else:
                    dw_dma.append((t, src))
                dw_tiles[(a, sub)] = (t, ds)

        def dw_ap(d, ct, ot):
            a, sub = d % NA, (d // 16)
            t, ds = dw_tiles[(a, sub)]
            return t[:, ds.index(d), ct, ot, :]

        bias_t = cpool.tile([128, 2], DT.float32)
        nc.scalar.dma_start(bias_t[:], bias_d[:])

        for j in range(nchunk):
            t0 = NT * j
            # one tile per chunk group: all batches' chunks for window j.
            # copy a's local col i holds xpad col t0+a+i; the matmul for
            # dense position d streams copy a=d%4 at local offset d-a
            # (multiple of 4 -> 8B-aligned for bf16).
            xc = xpool.tile([128, bpc, NA, 2, CH_W], DT.bfloat16)
            # a-major issue order: all batches' copy a land before copy a+1,
            # so phase j=0's early dense positions (d = a mod 4) can start
            # while later copies stream in
            for a in range(NA):
                for b in range(bpc):
                    lo_real = max(0, PAD - a - t0)
                    hi_real = min(CH_W, Lc - t0 + PAD - a)
                    if lo_real:
                        nc.scalar.dma_start(
                            xc[:, b, a, :, 0:lo_real], zp_d[:, :, 0:lo_real]
                        )
                    if hi_real < CH_W:
                        nc.scalar.dma_start(
                            xc[:, b, a, :, hi_real:CH_W],
                            zp_d[:, :, 0 : CH_W - hi_real],
                        )
                    for ct in range(2):
                        nc.scalar.dma_start(
                            xc[:, b, a, ct, lo_real:hi_real],
                            x_d[
                                b,
                                ct * 128 : (ct + 1) * 128,
                                t0 + a - PAD + lo_real : t0 + a - PAD + hi_real,
                            ],
                        )
            if j == 0:
                for t, src in dw_dma:
                    nc.scalar.dma_start(t[:], src)
            d_order = [d for a in range(NA) for d in range(a, ND, NA)]
            for ot in range(2):
                # one 4-bank tile; batch b's accumulation group lives in its
                # own PSUM bank (slice [:, b, :]).  The very last phase is
                # split into two batch-pair sub-phases so its drain + output
                # DMA pipeline against the second sub-phase's matmuls.
                last_phase = j == nchunk - 1 and ot == 1
                bgroups = [(0, 1), (2, 3)] if last_phase and bpc == 4 else [tuple(range(bpc))]
                ps = pspool.tile([128, bpc, NT], DT.float32, name="ps", tag="ps")
                for bg in bgroups:
                    n_acc = ND * 2
                    i = 0
                    for d in d_order:
                        a = d % NA
                        off = d - a
                        for ct in range(2):
                            w_ap = dw_ap(d, ct, ot)
                            for b in bg:
                                nc.tensor.matmul(
                                    ps[:, b, :],
                                    w_ap,
                                    xc[:, b, a, ct, off : off + NT],
                                    start=(i == 0),
                                    stop=(i == n_acc - 1),
                                )
                            i += 1
                    for b in bg:
                        ob = opool.tile([128, NT], DT.float32)
                        nc.vector.tensor_scalar_add(
                            ob[:], ps[:, b, :], bias_t[:, ot : ot + 1]
                        )
                        eng = nc.gpsimd if b % 2 == 0 else nc.sync
                        eng.dma_start(
                            y_d[b, ot * 128 : (ot + 1) * 128, t0 : t0 + NT], ob[:]
                        )

    nc.compile()
    return nc


def make_inputs(input, weight, P, bias):
    """Host-side prep shared by kernel() and the timing harness."""
    D = build_dense_kernel(weight, P)[:, :, :ND]  # [O, C, ND]
    # D axes: [ot, o, ct, c, d] -> dw[c, d, ct, ot, o]
    dw = np.ascontiguousarray(
        to_bf16(D).reshape(2, 128, 2, 128, ND).transpose(3, 4, 2, 0, 1)
    )
    bias2 = np.ascontiguousarray(np.asarray(bias, np.float32).reshape(2, 128).T)
    xq = to_bf16(np.ascontiguousarray(input, np.float32))  # [B, C, L] bf16
    zp = np.zeros((128, 2, 32), np.uint16).view(mybir.dt.np(DT.bfloat16))
    return xq, dw, bias2, zp, D


def tail_col(input, D, bias):
    """Output col t=L (the one past the last full chunk): only taps d < PAD
    see real data. Computed host-side (32x256 values, 0.02% of the FLOPs)."""
    xt = np.ascontiguousarray(input[:, :, L - PAD :], np.float32)  # [B, C, PAD]
    Dt = np.ascontiguousarray(D[:, :, :PAD], np.float32)  # [O, C, PAD]
    t = Dt.reshape(COUT, -1) @ xt.reshape(B, -1).T  # [O, B]
    return t.T + np.asarray(bias, np.float32)[None, :]  # [B, O]


def kernel(input, weight, P, bias):
    input = np.ascontiguousarray(input, np.float32)
    xq, dw, bias2, zp, D = make_inputs(input, weight, P, bias)

    if "nc" not in _nc_cache:
        _nc_cache["nc"] = build_nc()
    nc = _nc_cache["nc"]

    in_maps = [
        {
            "x": np.ascontiguousarray(xq[i * BPC : (i + 1) * BPC]),
            "dw": dw,
            "bias": bias2,
            "zp": zp,
        }
        for i in range(NCORES)
    ]
    res = run_bass_kernel_spmd(nc, in_maps, core_ids=list(range(NCORES)))
    out = np.empty((B, COUT, TOUT), np.float32)
    out[:, :, :L] = np.concatenate([r["y"] for r in res.results], axis=0)
    out[:, :, L] = tail_col(input, D, bias)
    return out
